# revision 16
# baseline (speedup 1.0000x reference)
"""Trainium2 Bass kernel for nn_MessagePassingLayer (bipartite GNN attention
message passing), distributed over 8 NeuronCores.

v5 design notes:
  - Node tables row-sharded 8 ways (inv 6250->6272 padded, asset 1250->1280).
  - dma_gather on TRN2 is descriptor-generation bound (~5.9ns/idx + 2.6us/call,
    independent of row bytes), so K|V are packed into one fp32 [N,512] row
    (one gather per edge total) and Q is never gathered: per target block the
    128 Q rows are densely loaded and Qe is formed on the PE with a
    host-precomputed transposed one-hot (exact in bf16).
  - Phase A: project local shard; FF runs feature-major off a host-supplied
    transposed bf16 copy of h, so there are no on-device transposes.  Small
    asset tables (kv_a fp32, q_a bf16) are AllGathered; the 51MB investor
    tables stay local (dir2 is shard-by-source + ReduceScatter of the segment
    stats instead).
  - Phase B per 128-edge chunk: gather KV[src]; Qe = oh_te @ Qblk (PE);
    qk = Qe*Ke (DVE, fp32); per-head reduce; ex = nw*exp(s/8) (max-term
    dropped -- exact to 1e-10 rel here: ex equals the reference's exactly
    when the segment max < 0, and the 1e-10 eps is negligible otherwise);
    exe=[ex*V | ex] in bf16; numer|sumex accumulated per target block by one
    one-hot matmul into fp32 PSUM; divide at block end.  Batched DVE ops
    amortize op overhead across 8-chunk gather batches.
  - dir2 partial numer/sumex (over all 10240 asset rows) are ReduceScattered.
  - Phase C: out = FF([h, msg]); msg transposed via hardware DMA-transpose
    (bf16); matmuls bf16, final gelu fp32 out.
  - gelu and exp live in different ACT table sets; an explicit dep keeps all
    phase-C gelus after the last phase-B exp to avoid table-reload thrash.
"""

import ml_dtypes
import numpy as np
from contextlib import ExitStack

import concourse.bass as bass
import concourse.tile as tile
from concourse.tile import add_dep_helper
from concourse import bacc, mybir
from concourse import bass_utils

F32 = mybir.dt.float32
BF16 = mybir.dt.bfloat16
I16 = mybir.dt.int16
AF = mybir.ActivationFunctionType
ALU = mybir.AluOpType

I_N, A_N, E_N = 50000, 10000, 200000
D, H, DK = 256, 4, 64
D2 = 2 * D
NC = 8
P = 128
ISH, ASH = I_N // NC, A_N // NC            # 6250, 1250
ISHP, ASHP = 6272, 1280
NT_I, NT_A = ISHP // P, ASHP // P          # 49, 10
NB2 = ASHP * NC // P                       # 80 global asset blocks
SUBB = 8                                   # chunks per gather call (1024 idx cap)

_LAST_EXEC_NS = None


# ----------------------------------------------------------------------------
# Host-side edge preparation
# ----------------------------------------------------------------------------

def _bucket(core, blk, n_blocks, srcidx, t128_all, nw):
    buckets = {}
    for c in range(NC):
        m_c = core == c
        for b in range(n_blocks):
            buckets[(c, b)] = np.nonzero(m_c & (blk == b))[0]
    meta = [max((len(buckets[(c, b)]) + P - 1) // P for c in range(NC))
            for b in range(n_blocks)]
    C = sum(meta)
    cores_out = []
    for c in range(NC):
        s16 = np.zeros(C * P, np.int64)
        t128 = np.full(C * P, -1, np.int64)
        nwv = np.zeros(C * P, np.float32)
        pos = 0
        for b in range(n_blocks):
            idx = buckets[(c, b)]
            n = len(idx)
            if meta[b] == 0:
                continue
            sl = slice(pos, pos + n)
            s16[sl] = srcidx[idx]
            t128[sl] = t128_all[idx]
            nwv[sl] = nw[idx]
            pos += meta[b] * P
        assert pos == C * P
        cores_out.append((s16, t128, nwv))
    return meta, C, cores_out


def _wrap16(flat_idx, C):
    assert flat_idx.max(initial=0) < 32768 and flat_idx.min(initial=0) >= 0
    w = flat_idx.astype(np.int16).reshape(C * 8, 16).T
    return np.tile(w, (8, 1)).copy()


def _colmajor(flat, C, rep=1):
    a = flat.reshape(C, P).T.copy()
    if rep == 1:
        return a
    return np.repeat(a[:, :, None], rep, axis=2).copy()


def _onehots(t128, C):
    """[C*128] targets (-1=pad) -> combined [C, 128, 256] = [oh_te | oh_et]."""
    bf = ml_dtypes.bfloat16
    t = t128.reshape(C, P, 1)
    oh_et = (t == np.arange(P).reshape(1, 1, P)).astype(bf)
    out = np.empty((C, P, 2 * P), bf)
    out[:, :, 0:P] = oh_et.transpose(0, 2, 1)
    out[:, :, P:2 * P] = oh_et
    return out


def _pad_rows(a, n):
    out = np.zeros((n, a.shape[1]), a.dtype)
    out[: a.shape[0]] = a
    return out


def _hT(h_pad, ntiles):
    """[N,256] f32 -> bf16 transposed-tile layout [N,256]:
    row (t*128+f), col (k*128+n) = h[t*128+n, k*128+f]."""
    bf = ml_dtypes.bfloat16
    x = h_pad.reshape(ntiles, P, 2, P)          # (t, n, k, f)
    x = x.transpose(0, 3, 2, 1)                 # (t, f, k, n)
    return np.ascontiguousarray(x.reshape(ntiles * P, D).astype(bf))


def _chunk_info(meta):
    info = []
    for b, k in enumerate(meta):
        for j in range(k):
            info.append((b, j == 0, j == k - 1))
    return info


# ----------------------------------------------------------------------------
# Device program
# ----------------------------------------------------------------------------

def _build(meta1, C1, meta2, C2):
    nc = bacc.Bacc("TRN2", target_bir_lowering=False, debug=False,
                   enable_asserts=True, num_devices=NC)

    hT_i_in = nc.dram_tensor("hT_i_in", [NT_I * P, D], BF16, kind="ExternalInput")
    hT_a_in = nc.dram_tensor("hT_a_in", [NT_A * P, D], BF16, kind="ExternalInput")
    w_m1 = nc.dram_tensor("w_m1", [2, P, D], BF16, kind="ExternalInput")
    w_m2 = nc.dram_tensor("w_m2", [2, P, D], BF16, kind="ExternalInput")
    w_qk = nc.dram_tensor("w_qk", [2, P, D2], BF16, kind="ExternalInput")
    w_v = nc.dram_tensor("w_v", [2, P, D], BF16, kind="ExternalInput")
    w_u1 = nc.dram_tensor("w_u1", [4, P, D], BF16, kind="ExternalInput")
    w_u2 = nc.dram_tensor("w_u2", [2, P, D], BF16, kind="ExternalInput")
    b_m1 = nc.dram_tensor("b_m1", [P, 2], F32, kind="ExternalInput")
    b_m2 = nc.dram_tensor("b_m2", [P, 2], F32, kind="ExternalInput")
    b_u1 = nc.dram_tensor("b_u1", [P, 2], F32, kind="ExternalInput")
    b_u2r = nc.dram_tensor("b_u2r", [1, D], BF16, kind="ExternalInput")

    d1_src = nc.dram_tensor("d1_src", [P, C1 * 8], I16, kind="ExternalInput")
    d1_nw = nc.dram_tensor("d1_nw", [P, C1, H], F32, kind="ExternalInput")
    d1_oh = nc.dram_tensor("d1_oh", [C1, P, 2 * P], BF16, kind="ExternalInput")
    d2_src = nc.dram_tensor("d2_src", [P, C2 * 8], I16, kind="ExternalInput")
    d2_nw = nc.dram_tensor("d2_nw", [P, C2, H], F32, kind="ExternalInput")
    d2_oh = nc.dram_tensor("d2_oh", [C2, P, 2 * P], BF16, kind="ExternalInput")

    out_inv = nc.dram_tensor("out_inv", [ISHP, D], F32, kind="ExternalOutput")
    out_ast = nc.dram_tensor("out_ast", [ASHP, D], F32, kind="ExternalOutput")

    info1 = _chunk_info(meta1)
    info2 = _chunk_info(meta2)

    with tile.TileContext(nc) as tc:
        with ExitStack() as ctx:
            wpool = ctx.enter_context(tc.tile_pool(name="w", bufs=1))
            hp = ctx.enter_context(tc.tile_pool(name="hp", bufs=3))
            tp = ctx.enter_context(tc.tile_pool(name="tp", bufs=3))
            op = ctx.enter_context(tc.tile_pool(name="op", bufs=3))
            gp = ctx.enter_context(tc.tile_pool(name="gp", bufs=2))
            sp = ctx.enter_context(tc.tile_pool(name="sp", bufs=2))
            ohp = ctx.enter_context(tc.tile_pool(name="ohp", bufs=4))
            qbp = ctx.enter_context(tc.tile_pool(name="qbp", bufs=2))
            ps_mm = ctx.enter_context(tc.tile_pool(name="ps_mm", bufs=2, space="PSUM"))
            ps_c = ctx.enter_context(tc.tile_pool(name="ps_c", bufs=2, space="PSUM"))
            ps_nu = ctx.enter_context(tc.tile_pool(name="ps_nu", bufs=2, space="PSUM"))
            ps_qe = ctx.enter_context(tc.tile_pool(name="ps_qe", bufs=2, space="PSUM"))
            dram = ctx.enter_context(tc.tile_pool(name="dram", bufs=1, space="DRAM"))

            ones_t = wpool.tile([1, P], BF16, tag="ones_t")
            nc.vector.memset(ones_t[:], 1.0)
            z256 = wpool.tile([P, D], F32, tag="z256")
            nc.vector.memset(z256[:], 0.0)
            z4 = wpool.tile([P, H], F32, tag="z4")
            nc.vector.memset(z4[:], 0.0)
            z256b = wpool.tile([P, D], BF16, tag="z256b")
            nc.vector.memset(z256b[:], 0.0)

            def load_w(dram_w, nk, nd, dt, tag):
                t = wpool.tile([P, nk, nd], dt, tag=tag)
                for k in range(nk):
                    nc.sync.dma_start(t[:, k, :], dram_w[k, :, :])
                return t

            m1_t = load_w(w_m1, 2, D, BF16, "wm1")
            m2_t = load_w(w_m2, 2, D, BF16, "wm2")
            qk_t = load_w(w_qk, 2, D2, BF16, "wqk")
            v_t = load_w(w_v, 2, D, BF16, "wv")
            u1_t = load_w(w_u1, 4, D, BF16, "wu1")
            u2_t = load_w(w_u2, 2, D, BF16, "wu2")

            def load_b(dram_b, tag):
                t = wpool.tile([P, 2], F32, tag=tag)
                nc.sync.dma_start(t[:], dram_b[:, :])
                return t

            bm1_t, bm2_t, bu1_t = load_b(b_m1, "bm1"), load_b(b_m2, "bm2"), load_b(b_u1, "bu1")
            bu2_t = wpool.tile([1, D], BF16, tag="bu2")
            nc.sync.dma_start(bu2_t[:], b_u2r[:, :])

            q_inv = dram.tile([ISHP, D], BF16, tag="q_inv")
            kv_inv = dram.tile([ISHP, D2], BF16, tag="kv_inv")
            q_a_sh = dram.tile([ASHP, D], BF16, tag="q_a_sh")
            kv_a_sh = dram.tile([ASHP, D2], BF16, tag="kv_a_sh")
            q_a_full = dram.tile([ASHP * NC, D], BF16, tag="q_a_full")
            kv_a_full = dram.tile([ASHP * NC, D2], BF16, tag="kv_a_full")
            numer_d = dram.tile([ASHP * NC, D], F32, tag="numer_d")
            sumex_d = dram.tile([ASHP * NC, H], F32, tag="sumex_d")
            numer_sh = dram.tile([ASHP, D], F32, tag="numer_sh")
            sumex_sh = dram.tile([ASHP, H], F32, tag="sumex_sh")
            msg_inv = dram.tile([ISHP, D], BF16, tag="msg_inv")
            msg_ast = dram.tile([ASHP, D], BF16, tag="msg_ast")

            # ================= Phase A =================
            def phase_a(hT_in, ntiles, q_dram, kv_dram):
                for t in range(ntiles):
                    hTb = tp.tile([P, 2, P], BF16, tag="hTb")
                    nc.sync.dma_start(hTb[:], hT_in[t * P:(t + 1) * P, :])

                    def ff_layer(inT, w_tile, b_tile, tag):
                        outT = tp.tile([P, 2, P], BF16, tag=tag)
                        for hf in range(2):
                            pm = ps_mm.tile([P, D2], F32, tag="pm")
                            for k in range(2):
                                nc.tensor.matmul(
                                    pm[:, 0:P],
                                    lhsT=w_tile[:, k, hf * P:(hf + 1) * P],
                                    rhs=inT[:, k, :],
                                    start=(k == 0), stop=(k == 1))
                            nc.scalar.activation(
                                outT[:, hf, :], pm[:, 0:P], AF.Gelu,
                                bias=b_tile[:, hf:hf + 1])
                        return outT

                    mT1 = ff_layer(hTb, m1_t, bm1_t, "mT1")
                    mT2 = ff_layer(mT1, m2_t, bm2_t, "mT2")

                    pqk = ps_mm.tile([P, D2], F32, tag="pm")
                    for k in range(2):
                        nc.tensor.matmul(pqk[:], lhsT=hTb[:, k, :],
                                         rhs=qk_t[:, k, :],
                                         start=(k == 0), stop=(k == 1))
                    oq = op.tile([P, D], BF16, tag="proj_q")
                    nc.vector.tensor_copy(oq[:], pqk[:, 0:D])
                    nc.sync.dma_start(q_dram[t * P:(t + 1) * P, :], oq[:])
                    ok_ = op.tile([P, D], BF16, tag="proj_k")
                    nc.vector.tensor_copy(ok_[:], pqk[:, D:D2])
                    nc.sync.dma_start(kv_dram[t * P:(t + 1) * P, 0:D], ok_[:])
                    pv = ps_mm.tile([P, D2], F32, tag="pm")
                    for k in range(2):
                        nc.tensor.matmul(pv[:, 0:D], lhsT=mT2[:, k, :],
                                         rhs=v_t[:, k, :],
                                         start=(k == 0), stop=(k == 1))
                    ov = op.tile([P, D], BF16, tag="proj_v")
                    nc.vector.tensor_copy(ov[:], pv[:, 0:D])
                    nc.sync.dma_start(kv_dram[t * P:(t + 1) * P, D:D2], ov[:])

            phase_a(hT_a_in, NT_A, q_a_sh, kv_a_sh)

            rg = [list(range(NC))]
            nc.gpsimd.collective_compute(
                "AllGather", ALU.bypass, replica_groups=rg,
                ins=[kv_a_sh.opt()], outs=[kv_a_full.opt()])
            nc.gpsimd.collective_compute(
                "AllGather", ALU.bypass, replica_groups=rg,
                ins=[q_a_sh.opt()], outs=[q_a_full.opt()])

            phase_a(hT_i_in, NT_I, q_inv, kv_inv)

            # ================= Phase B =================
            last_exp = [None]

            def phase_b(info, C, meta, src_sb, nw_sb, oh_dram,
                        q_tbl, kv_tbl, sink):
                numer = [None]
                qblk = None

                def flush(batch):
                    g0, n, ohg, exe = batch
                    for j in range(n):
                        c = g0 + j
                        blk, first, last = info[c]
                        if first:
                            numer[0] = ps_nu.tile([P, D + H], F32, tag="nu", name="nu")
                        nu_ps = numer[0]
                        nc.tensor.matmul(nu_ps[:], lhsT=ohg[:, j, P:2 * P],
                                         rhs=exe[:, j, :],
                                         start=first, stop=last)
                        if last:
                            rows = slice(blk * P, (blk + 1) * P)
                            if sink[0] == "msg":
                                den = sp.tile([P, H], F32, tag="den")
                                nc.vector.tensor_scalar(
                                    den[:], nu_ps[:, D:D + H], 1e-10, None,
                                    ALU.add)
                                rec = sp.tile([P, H], F32, tag="rec")
                                nc.vector.reciprocal(rec[:], den[:])
                                msg = op.tile([P, D], BF16, tag="msg")
                                nc.vector.tensor_tensor(
                                    msg[:].rearrange("p (h k) -> p h k", h=H),
                                    nu_ps[:, 0:D].rearrange("p (h k) -> p h k", h=H),
                                    rec[:].unsqueeze(-1).to_broadcast((P, H, DK)),
                                    ALU.mult)
                                nc.sync.dma_start(sink[1][rows, :], msg[:])
                            else:
                                nu_sb = op.tile([P, D], F32, tag="nu_sb")
                                nc.vector.tensor_copy(nu_sb[:], nu_ps[:, 0:D])
                                nc.sync.dma_start(sink[1][rows, :], nu_sb[:])
                                se_sb = sp.tile([P, H], F32, tag="se_sb")
                                nc.vector.tensor_copy(se_sb[:], nu_ps[:, D:D + H])
                                nc.sync.dma_start(sink[2][rows, :], se_sb[:])

                pend = None
                for g0 in range(0, C, SUBB):
                    n = min(SUBB, C - g0)
                    kvg = gp.tile([P, SUBB, D2], BF16, tag="kvg", bufs=4)
                    nc.gpsimd.dma_gather(
                        out_ap=kvg[:, 0:n, :], in_ap=kv_tbl,
                        idxs_ap=src_sb[:, g0 * 8:(g0 + n) * 8],
                        num_idxs=n * P, num_idxs_reg=n * P, elem_size=D2)
                    ohg = ohp.tile([P, SUBB, 2 * P], BF16, tag="ohg")
                    nc.sync.dma_start(
                        ohg[:, 0:n, :],
                        oh_dram[g0:g0 + n].transpose([1, 0, 2]))
                    qk = sp.tile([P, SUBB, D], F32, tag="qk", bufs=3)
                    for j in range(n):
                        c = g0 + j
                        blk, first, last = info[c]
                        if first:
                            qblk = qbp.tile([P, D], BF16, tag="qblk")
                            nc.sync.dma_start(
                                qblk[:], q_tbl[blk * P:(blk + 1) * P, :])
                        qe = ps_qe.tile([P, D], F32, tag="qe")
                        nc.tensor.matmul(qe[:], lhsT=ohg[:, j, 0:P], rhs=qblk[:],
                                         start=True, stop=True)
                        nc.vector.tensor_tensor(
                            qk[:, j, :], qe[:], kvg[:, j, 0:D], ALU.mult)
                    s4 = sp.tile([P, SUBB, H], F32, tag="s4")
                    nc.vector.tensor_reduce(
                        s4[:, 0:n, :],
                        qk[:, 0:n, :].rearrange("p c (h k) -> p c h k", h=H),
                        axis=mybir.AxisListType.X, op=ALU.add)
                    ex0 = sp.tile([P, SUBB, H], F32, tag="ex0")
                    last_exp[0] = nc.scalar.activation(
                        ex0[:, 0:n, :], s4[:, 0:n, :], AF.Exp, scale=0.125)
                    exb = sp.tile([P, SUBB, H], F32, tag="exb")
                    nc.vector.tensor_tensor(
                        exb[:, 0:n, :], ex0[:, 0:n, :], nw_sb[:, g0:g0 + n, :],
                        ALU.mult)
                    exe = sp.tile([P, SUBB, D + H], BF16, tag="exe", bufs=3)
                    nc.vector.tensor_tensor(
                        exe[:, 0:n, 0:D].rearrange("p c (h k) -> p c h k", h=H),
                        kvg[:, 0:n, D:D2].rearrange("p c (h k) -> p c h k", h=H),
                        exb[:, 0:n, :].unsqueeze(-1).to_broadcast((P, n, H, DK)),
                        ALU.mult)
                    nc.vector.tensor_copy(exe[:, 0:n, D:D + H], exb[:, 0:n, :])
                    if pend is not None:
                        flush(pend)
                    pend = (g0, n, ohg, exe)
                if pend is not None:
                    flush(pend)
                for b, k in enumerate(meta):
                    if k != 0:
                        continue
                    rows = slice(b * P, (b + 1) * P)
                    if sink[0] == "msg":
                        nc.sync.dma_start(sink[1][rows, :], z256b[:])
                    else:
                        nc.sync.dma_start(sink[1][rows, :], z256[:])
                        nc.sync.dma_start(sink[2][rows, :], z4[:])

            d1_src_sb = wpool.tile([P, C1 * 8], I16, tag="d1_src_sb")
            nc.sync.dma_start(d1_src_sb[:], d1_src[:, :])
            d1_nw_sb = wpool.tile([P, C1, H], F32, tag="d1_nw_sb")
            nc.sync.dma_start(d1_nw_sb[:], d1_nw[:, :, :])
            d2_src_sb = wpool.tile([P, C2 * 8], I16, tag="d2_src_sb")
            nc.sync.dma_start(d2_src_sb[:], d2_src[:, :])
            d2_nw_sb = wpool.tile([P, C2, H], F32, tag="d2_nw_sb")
            nc.sync.dma_start(d2_nw_sb[:], d2_nw[:, :, :])

            phase_b(info1, C1, meta1, d1_src_sb, d1_nw_sb, d1_oh,
                    q_inv[:, :], kv_a_full[:, :], ("msg", msg_inv))
            phase_b(info2, C2, meta2, d2_src_sb, d2_nw_sb, d2_oh,
                    q_a_full[:, :], kv_inv[:, :], ("acc", numer_d, sumex_d))

            nc.gpsimd.collective_compute(
                "ReduceScatter", ALU.add, replica_groups=rg,
                ins=[numer_d.opt()], outs=[numer_sh.opt()])
            nc.gpsimd.collective_compute(
                "ReduceScatter", ALU.add, replica_groups=rg,
                ins=[sumex_d.opt()], outs=[sumex_sh.opt()])

            # ================= Phase C =================
            first_gelu = [None]

            def phase_c(ntiles, hT_in, msg_dram, out_dram):
                for t in range(ntiles):
                    cat = tp.tile([P, 4, P], BF16, tag="cat")
                    nc.sync.dma_start(cat[:, 0:2, :], hT_in[t * P:(t + 1) * P, :])
                    nc.sync.dma_start_transpose(
                        cat[:, 2, :], msg_dram[t * P:(t + 1) * P, 0:P])
                    nc.scalar.dma_start_transpose(
                        cat[:, 3, :], msg_dram[t * P:(t + 1) * P, P:D])
                    y1 = tp.tile([P, 2, P], BF16, tag="y1")
                    pm = ps_c.tile([P, D], F32, tag="pmc")
                    for hf in range(2):
                        for k in range(4):
                            nc.tensor.matmul(
                                pm[:, hf * P:(hf + 1) * P],
                                lhsT=u1_t[:, k, hf * P:(hf + 1) * P],
                                rhs=cat[:, k, :],
                                start=(k == 0), stop=(k == 3))
                    for hf in range(2):
                        g = nc.scalar.activation(
                            y1[:, hf, :], pm[:, hf * P:(hf + 1) * P], AF.Gelu,
                            bias=bu1_t[:, hf:hf + 1])
                        if first_gelu[0] is None:
                            first_gelu[0] = g
                            if last_exp[0] is not None:
                                add_dep_helper(
                                    g.ins, last_exp[0].ins,
                                    reason="gelu after exp (ACT tables)")
                    po = ps_c.tile([P, D], F32, tag="pmc")
                    for k in range(2):
                        nc.tensor.matmul(po[:], lhsT=y1[:, k, :],
                                         rhs=u2_t[:, k, :],
                                         start=(k == 0), stop=False)
                    nc.tensor.matmul(po[:], lhsT=ones_t[0:1, :],
                                     rhs=bu2_t[0:1, :], start=False, stop=True)
                    ot = op.tile([P, D], F32, tag="fin")
                    nc.scalar.activation(ot[:], po[:], AF.Gelu)
                    nc.sync.dma_start(out_dram[t * P:(t + 1) * P, :], ot[:])

            phase_c(NT_I, hT_i_in, msg_inv, out_inv)

            for t in range(NT_A):
                nu = hp.tile([P, D], F32, tag="nu_f")
                nc.sync.dma_start(nu[:], numer_sh[t * P:(t + 1) * P, :])
                se = sp.tile([P, H], F32, tag="se_f")
                nc.sync.dma_start(se[:], sumex_sh[t * P:(t + 1) * P, :])
                den = sp.tile([P, H], F32, tag="den")
                nc.vector.tensor_scalar(den[:], se[:], 1e-10, None, ALU.add)
                rec = sp.tile([P, H], F32, tag="rec")
                nc.vector.reciprocal(rec[:], den[:])
                msg = op.tile([P, D], BF16, tag="msg")
                nc.vector.tensor_tensor(
                    msg[:].rearrange("p (h k) -> p h k", h=H),
                    nu[:].rearrange("p (h k) -> p h k", h=H),
                    rec[:].unsqueeze(-1).to_broadcast((P, H, DK)), ALU.mult)
                nc.sync.dma_start(msg_ast[t * P:(t + 1) * P, :], msg[:])

            phase_c(NT_A, hT_a_in, msg_ast, out_ast)

    nc.compile()
    return nc


# ----------------------------------------------------------------------------
# Entry point
# ----------------------------------------------------------------------------

def kernel(inv_h, asset_h, inv_norm_w, asset_norm_w,
           m_w1, m_b1, m_w2, m_b2, Wq, Wk, Wv,
           u_w1, u_b1, u_w2, u_b2, edge_tgt, edge_src):
    global _LAST_EXEC_NS
    bf = ml_dtypes.bfloat16
    inv_h = np.asarray(inv_h, np.float32)
    asset_h = np.asarray(asset_h, np.float32)
    inv_norm_w = np.asarray(inv_norm_w, np.float32)
    asset_norm_w = np.asarray(asset_norm_w, np.float32)
    edge_tgt = np.asarray(edge_tgt).astype(np.int64)
    edge_src = np.asarray(edge_src).astype(np.int64)
    m_w1, m_b1 = np.asarray(m_w1, np.float32), np.asarray(m_b1, np.float32)
    m_w2, m_b2 = np.asarray(m_w2, np.float32), np.asarray(m_b2, np.float32)
    Wq, Wk, Wv = (np.asarray(x, np.float32) for x in (Wq, Wk, Wv))
    u_w1, u_b1 = np.asarray(u_w1, np.float32), np.asarray(u_b1, np.float32)
    u_w2, u_b2 = np.asarray(u_w2, np.float32), np.asarray(u_b2, np.float32)

    ast_row = (edge_src // ASH) * ASHP + (edge_src % ASH)
    inv_core = edge_tgt // ISH
    inv_loc = edge_tgt - inv_core * ISH

    meta1, C1, d1 = _bucket(
        core=inv_core, blk=inv_loc // P, n_blocks=NT_I,
        srcidx=ast_row, t128_all=inv_loc % P, nw=inv_norm_w)
    meta2, C2, d2 = _bucket(
        core=inv_core, blk=ast_row // P, n_blocks=NB2,
        srcidx=inv_loc, t128_all=ast_row % P, nw=asset_norm_w)

    nc = _build(meta1, C1, meta2, C2)

    w_qk_h = np.concatenate([Wq.reshape(2, P, D), Wk.reshape(2, P, D)], axis=2)
    common = {
        "w_m1": m_w1.reshape(2, P, D).astype(bf),
        "w_m2": m_w2.reshape(2, P, D).astype(bf),
        "w_qk": w_qk_h.astype(bf),
        "w_v": Wv.reshape(2, P, D).astype(bf),
        "w_u1": u_w1.reshape(4, P, D).astype(bf),
        "w_u2": u_w2.reshape(2, P, D).astype(bf),
        "b_m1": m_b1.reshape(2, P).T.copy(),
        "b_m2": m_b2.reshape(2, P).T.copy(),
        "b_u1": u_b1.reshape(2, P).T.copy(),
        "b_u2r": u_b2.reshape(1, D).astype(bf),
    }

    in_maps = []
    for c in range(NC):
        s1, t1, n1 = d1[c]
        s2, t2, n2 = d2[c]
        m = dict(common)
        m["hT_i_in"] = _hT(_pad_rows(inv_h[c * ISH:(c + 1) * ISH], ISHP), NT_I)
        m["hT_a_in"] = _hT(_pad_rows(asset_h[c * ASH:(c + 1) * ASH], ASHP), NT_A)
        m["d1_src"] = _wrap16(s1, C1)
        m["d1_nw"] = _colmajor(n1, C1, rep=H)
        m["d1_oh"] = _onehots(t1, C1)
        m["d2_src"] = _wrap16(s2, C2)
        m["d2_nw"] = _colmajor(n2, C2, rep=H)
        m["d2_oh"] = _onehots(t2, C2)
        in_maps.append(m)

    res = bass_utils.run_bass_kernel_spmd(
        nc, in_maps, core_ids=list(range(NC)), trace=True)
    _LAST_EXEC_NS = res.exec_time_ns

    inv_out = np.concatenate(
        [res.results[c]["out_inv"][:ISH] for c in range(NC)], axis=0)
    ast_out = np.concatenate(
        [res.results[c]["out_ast"][:ASH] for c in range(NC)], axis=0)
    return inv_out, ast_out


# revision 17
# speedup vs baseline: 1.0390x; 1.0390x over previous
"""Trainium2 Bass kernel for nn_MessagePassingLayer (bipartite GNN attention
message passing), distributed over 8 NeuronCores.

v5 design notes:
  - Node tables row-sharded 8 ways (inv 6250->6272 padded, asset 1250->1280).
  - dma_gather on TRN2 is descriptor-generation bound (~5.9ns/idx + 2.6us/call,
    independent of row bytes), so K|V are packed into one fp32 [N,512] row
    (one gather per edge total) and Q is never gathered: per target block the
    128 Q rows are densely loaded and Qe is formed on the PE with a
    host-precomputed transposed one-hot (exact in bf16).
  - Phase A: project local shard; FF runs feature-major off a host-supplied
    transposed bf16 copy of h, so there are no on-device transposes.  Small
    asset tables (kv_a fp32, q_a bf16) are AllGathered; the 51MB investor
    tables stay local (dir2 is shard-by-source + ReduceScatter of the segment
    stats instead).
  - Phase B per 128-edge chunk: gather KV[src]; Qe = oh_te @ Qblk (PE);
    qk = Qe*Ke (DVE, fp32); per-head reduce; ex = nw*exp(s/8) (max-term
    dropped -- exact to 1e-10 rel here: ex equals the reference's exactly
    when the segment max < 0, and the 1e-10 eps is negligible otherwise);
    exe=[ex*V | ex] in bf16; numer|sumex accumulated per target block by one
    one-hot matmul into fp32 PSUM; divide at block end.  Batched DVE ops
    amortize op overhead across 8-chunk gather batches.
  - dir2 partial numer/sumex (over all 10240 asset rows) are ReduceScattered.
  - Phase C: out = FF([h, msg]); msg transposed via hardware DMA-transpose
    (bf16); matmuls bf16, final gelu fp32 out.
  - gelu and exp live in different ACT table sets; an explicit dep keeps all
    phase-C gelus after the last phase-B exp to avoid table-reload thrash.
"""

import ml_dtypes
import numpy as np
from contextlib import ExitStack

import concourse.bass as bass
import concourse.tile as tile
from concourse.tile import add_dep_helper
from concourse import bacc, mybir
from concourse import bass_utils

F32 = mybir.dt.float32
BF16 = mybir.dt.bfloat16
I16 = mybir.dt.int16
AF = mybir.ActivationFunctionType
ALU = mybir.AluOpType

I_N, A_N, E_N = 50000, 10000, 200000
D, H, DK = 256, 4, 64
D2 = 2 * D
NC = 8
P = 128
ISH, ASH = I_N // NC, A_N // NC            # 6250, 1250
ISHP, ASHP = 6272, 1280
NT_I, NT_A = ISHP // P, ASHP // P          # 49, 10
NB2 = ASHP * NC // P                       # 80 global asset blocks
SUBB = 8                                   # chunks per gather call (1024 idx cap)

_LAST_EXEC_NS = None


# ----------------------------------------------------------------------------
# Host-side edge preparation
# ----------------------------------------------------------------------------

def _bucket(core, blk, n_blocks, srcidx, t128_all, nw):
    buckets = {}
    for c in range(NC):
        m_c = core == c
        for b in range(n_blocks):
            buckets[(c, b)] = np.nonzero(m_c & (blk == b))[0]
    meta = [max((len(buckets[(c, b)]) + P - 1) // P for c in range(NC))
            for b in range(n_blocks)]
    C = sum(meta)
    cores_out = []
    for c in range(NC):
        s16 = np.zeros(C * P, np.int64)
        t128 = np.full(C * P, -1, np.int64)
        nwv = np.zeros(C * P, np.float32)
        pos = 0
        for b in range(n_blocks):
            idx = buckets[(c, b)]
            n = len(idx)
            if meta[b] == 0:
                continue
            sl = slice(pos, pos + n)
            s16[sl] = srcidx[idx]
            t128[sl] = t128_all[idx]
            nwv[sl] = nw[idx]
            pos += meta[b] * P
        assert pos == C * P
        cores_out.append((s16, t128, nwv))
    return meta, C, cores_out


def _wrap16(flat_idx, C):
    assert flat_idx.max(initial=0) < 32768 and flat_idx.min(initial=0) >= 0
    w = flat_idx.astype(np.int16).reshape(C * 8, 16).T
    return np.tile(w, (8, 1)).copy()


def _colmajor(flat, C, rep=1):
    a = flat.reshape(C, P).T.copy()
    if rep == 1:
        return a
    return np.repeat(a[:, :, None], rep, axis=2).copy()


def _onehots(t128, C):
    """[C*128] targets (-1=pad) -> combined [C, 128, 256] = [oh_te | oh_et]."""
    bf = ml_dtypes.bfloat16
    t = t128.reshape(C, P, 1)
    oh_et = (t == np.arange(P).reshape(1, 1, P)).astype(bf)
    out = np.empty((C, P, 2 * P), bf)
    out[:, :, 0:P] = oh_et.transpose(0, 2, 1)
    out[:, :, P:2 * P] = oh_et
    return out


def _pad_rows(a, n):
    out = np.zeros((n, a.shape[1]), a.dtype)
    out[: a.shape[0]] = a
    return out


def _hT(h_pad, ntiles):
    """[N,256] f32 -> bf16 transposed-tile layout [N,256]:
    row (t*128+f), col (k*128+n) = h[t*128+n, k*128+f]."""
    bf = ml_dtypes.bfloat16
    x = h_pad.reshape(ntiles, P, 2, P)          # (t, n, k, f)
    x = x.transpose(0, 3, 2, 1)                 # (t, f, k, n)
    return np.ascontiguousarray(x.reshape(ntiles * P, D).astype(bf))


def _chunk_info(meta):
    info = []
    for b, k in enumerate(meta):
        for j in range(k):
            info.append((b, j == 0, j == k - 1))
    return info


# ----------------------------------------------------------------------------
# Device program
# ----------------------------------------------------------------------------

def _build(meta1, C1, meta2, C2):
    nc = bacc.Bacc("TRN2", target_bir_lowering=False, debug=False,
                   enable_asserts=True, num_devices=NC)

    hT_i_in = nc.dram_tensor("hT_i_in", [NT_I * P, D], BF16, kind="ExternalInput")
    hT_a_in = nc.dram_tensor("hT_a_in", [NT_A * P, D], BF16, kind="ExternalInput")
    w_m1 = nc.dram_tensor("w_m1", [2, P, D], BF16, kind="ExternalInput")
    w_m2 = nc.dram_tensor("w_m2", [2, P, D], BF16, kind="ExternalInput")
    w_qk = nc.dram_tensor("w_qk", [2, P, D2], BF16, kind="ExternalInput")
    w_v = nc.dram_tensor("w_v", [2, P, D], BF16, kind="ExternalInput")
    w_u1 = nc.dram_tensor("w_u1", [4, P, D], BF16, kind="ExternalInput")
    w_u2 = nc.dram_tensor("w_u2", [2, P, D], BF16, kind="ExternalInput")
    b_m1 = nc.dram_tensor("b_m1", [P, 2], F32, kind="ExternalInput")
    b_m2 = nc.dram_tensor("b_m2", [P, 2], F32, kind="ExternalInput")
    b_u1 = nc.dram_tensor("b_u1", [P, 2], F32, kind="ExternalInput")
    b_u2r = nc.dram_tensor("b_u2r", [1, D], BF16, kind="ExternalInput")

    d1_src = nc.dram_tensor("d1_src", [P, C1 * 8], I16, kind="ExternalInput")
    d1_nw = nc.dram_tensor("d1_nw", [P, C1, H], F32, kind="ExternalInput")
    d1_oh = nc.dram_tensor("d1_oh", [C1, P, 2 * P], BF16, kind="ExternalInput")
    d2_src = nc.dram_tensor("d2_src", [P, C2 * 8], I16, kind="ExternalInput")
    d2_nw = nc.dram_tensor("d2_nw", [P, C2, H], F32, kind="ExternalInput")
    d2_oh = nc.dram_tensor("d2_oh", [C2, P, 2 * P], BF16, kind="ExternalInput")

    out_inv = nc.dram_tensor("out_inv", [ISHP, D], F32, kind="ExternalOutput")
    out_ast = nc.dram_tensor("out_ast", [ASHP, D], F32, kind="ExternalOutput")

    info1 = _chunk_info(meta1)
    info2 = _chunk_info(meta2)

    with tile.TileContext(nc) as tc:
        with ExitStack() as ctx:
            wpool = ctx.enter_context(tc.tile_pool(name="w", bufs=1))
            hp = ctx.enter_context(tc.tile_pool(name="hp", bufs=3))
            tp = ctx.enter_context(tc.tile_pool(name="tp", bufs=3))
            op = ctx.enter_context(tc.tile_pool(name="op", bufs=3))
            gp = ctx.enter_context(tc.tile_pool(name="gp", bufs=2))
            sp = ctx.enter_context(tc.tile_pool(name="sp", bufs=2))
            ohp = ctx.enter_context(tc.tile_pool(name="ohp", bufs=4))
            qbp = ctx.enter_context(tc.tile_pool(name="qbp", bufs=2))
            ps_mm = ctx.enter_context(tc.tile_pool(name="ps_mm", bufs=2, space="PSUM"))
            ps_c = ctx.enter_context(tc.tile_pool(name="ps_c", bufs=2, space="PSUM"))
            ps_nu = ctx.enter_context(tc.tile_pool(name="ps_nu", bufs=2, space="PSUM"))
            ps_qe = ctx.enter_context(tc.tile_pool(name="ps_qe", bufs=2, space="PSUM"))
            dram = ctx.enter_context(tc.tile_pool(name="dram", bufs=1, space="DRAM"))

            ones_t = wpool.tile([1, P], BF16, tag="ones_t")
            nc.vector.memset(ones_t[:], 1.0)
            z256 = wpool.tile([P, D], F32, tag="z256")
            nc.vector.memset(z256[:], 0.0)
            z4 = wpool.tile([P, H], F32, tag="z4")
            nc.vector.memset(z4[:], 0.0)
            z256b = wpool.tile([P, D], BF16, tag="z256b")
            nc.vector.memset(z256b[:], 0.0)

            def load_w(dram_w, nk, nd, dt, tag):
                t = wpool.tile([P, nk, nd], dt, tag=tag)
                for k in range(nk):
                    nc.sync.dma_start(t[:, k, :], dram_w[k, :, :])
                return t

            m1_t = load_w(w_m1, 2, D, BF16, "wm1")
            m2_t = load_w(w_m2, 2, D, BF16, "wm2")
            qk_t = load_w(w_qk, 2, D2, BF16, "wqk")
            v_t = load_w(w_v, 2, D, BF16, "wv")
            u1_t = load_w(w_u1, 4, D, BF16, "wu1")
            u2_t = load_w(w_u2, 2, D, BF16, "wu2")

            def load_b(dram_b, tag):
                t = wpool.tile([P, 2], F32, tag=tag)
                nc.sync.dma_start(t[:], dram_b[:, :])
                return t

            bm1_t, bm2_t, bu1_t = load_b(b_m1, "bm1"), load_b(b_m2, "bm2"), load_b(b_u1, "bu1")
            bu2_t = wpool.tile([1, D], BF16, tag="bu2")
            nc.sync.dma_start(bu2_t[:], b_u2r[:, :])

            q_inv = dram.tile([ISHP, D], BF16, tag="q_inv")
            kv_inv = dram.tile([ISHP, D2], BF16, tag="kv_inv")
            q_a_sh = dram.tile([ASHP, D], BF16, tag="q_a_sh")
            kv_a_sh = dram.tile([ASHP, D2], BF16, tag="kv_a_sh")
            q_a_full = dram.tile([ASHP * NC, D], BF16, tag="q_a_full")
            kv_a_full = dram.tile([ASHP * NC, D2], BF16, tag="kv_a_full")
            numer_d = dram.tile([ASHP * NC, D], F32, tag="numer_d")
            sumex_d = dram.tile([ASHP * NC, H], F32, tag="sumex_d")
            numer_sh = dram.tile([ASHP, D], F32, tag="numer_sh")
            sumex_sh = dram.tile([ASHP, H], F32, tag="sumex_sh")
            msg_inv = dram.tile([ISHP, D], BF16, tag="msg_inv")
            msg_ast = dram.tile([ASHP, D], BF16, tag="msg_ast")

            # ================= Phase A =================
            def phase_a(hT_in, ntiles, q_dram, kv_dram):
                for t in range(ntiles):
                    hTb = tp.tile([P, 2, P], BF16, tag="hTb")
                    nc.sync.dma_start(hTb[:], hT_in[t * P:(t + 1) * P, :])

                    def ff_layer(inT, w_tile, b_tile, tag):
                        outT = tp.tile([P, 2, P], BF16, tag=tag)
                        for hf in range(2):
                            pm = ps_mm.tile([P, D2], F32, tag="pm")
                            for k in range(2):
                                nc.tensor.matmul(
                                    pm[:, 0:P],
                                    lhsT=w_tile[:, k, hf * P:(hf + 1) * P],
                                    rhs=inT[:, k, :],
                                    start=(k == 0), stop=(k == 1))
                            nc.scalar.activation(
                                outT[:, hf, :], pm[:, 0:P], AF.Gelu,
                                bias=b_tile[:, hf:hf + 1])
                        return outT

                    mT1 = ff_layer(hTb, m1_t, bm1_t, "mT1")
                    mT2 = ff_layer(mT1, m2_t, bm2_t, "mT2")

                    pqk = ps_mm.tile([P, D2], F32, tag="pm")
                    for k in range(2):
                        nc.tensor.matmul(pqk[:], lhsT=hTb[:, k, :],
                                         rhs=qk_t[:, k, :],
                                         start=(k == 0), stop=(k == 1))
                    oq = op.tile([P, D], BF16, tag="proj_q")
                    nc.vector.tensor_copy(oq[:], pqk[:, 0:D])
                    nc.sync.dma_start(q_dram[t * P:(t + 1) * P, :], oq[:])
                    ok_ = op.tile([P, D], BF16, tag="proj_k")
                    nc.vector.tensor_copy(ok_[:], pqk[:, D:D2])
                    nc.sync.dma_start(kv_dram[t * P:(t + 1) * P, 0:D], ok_[:])
                    pv = ps_mm.tile([P, D2], F32, tag="pm")
                    for k in range(2):
                        nc.tensor.matmul(pv[:, 0:D], lhsT=mT2[:, k, :],
                                         rhs=v_t[:, k, :],
                                         start=(k == 0), stop=(k == 1))
                    ov = op.tile([P, D], BF16, tag="proj_v")
                    nc.vector.tensor_copy(ov[:], pv[:, 0:D])
                    nc.sync.dma_start(kv_dram[t * P:(t + 1) * P, D:D2], ov[:])

            phase_a(hT_a_in, NT_A, q_a_sh, kv_a_sh)

            rg = [list(range(NC))]
            nc.gpsimd.collective_compute(
                "AllGather", ALU.bypass, replica_groups=rg,
                ins=[kv_a_sh.opt()], outs=[kv_a_full.opt()])
            nc.gpsimd.collective_compute(
                "AllGather", ALU.bypass, replica_groups=rg,
                ins=[q_a_sh.opt()], outs=[q_a_full.opt()])

            phase_a(hT_i_in, NT_I, q_inv, kv_inv)

            # ================= Phase B =================
            last_exp = [None]

            def phase_b(info, C, meta, src_sb, nw_sb, oh_dram,
                        q_tbl, kv_tbl, sink):
                numer = [None]
                qblk = None

                def flush(batch):
                    g0, n, ohg, exe = batch
                    for j in range(n):
                        c = g0 + j
                        blk, first, last = info[c]
                        if first:
                            numer[0] = ps_nu.tile([P, D + H], F32, tag="nu", name="nu")
                        nu_ps = numer[0]
                        nc.tensor.matmul(nu_ps[:], lhsT=ohg[:, j, P:2 * P],
                                         rhs=exe[:, j, :],
                                         start=first, stop=last)
                        if last:
                            rows = slice(blk * P, (blk + 1) * P)
                            if sink[0] == "msg":
                                den = sp.tile([P, H], F32, tag="den")
                                nc.vector.tensor_scalar(
                                    den[:], nu_ps[:, D:D + H], 1e-10, None,
                                    ALU.add)
                                rec = sp.tile([P, H], F32, tag="rec")
                                nc.vector.reciprocal(rec[:], den[:])
                                msg = op.tile([P, D], BF16, tag="msg")
                                nc.vector.tensor_tensor(
                                    msg[:].rearrange("p (h k) -> p h k", h=H),
                                    nu_ps[:, 0:D].rearrange("p (h k) -> p h k", h=H),
                                    rec[:].unsqueeze(-1).to_broadcast((P, H, DK)),
                                    ALU.mult)
                                nc.sync.dma_start(sink[1][rows, :], msg[:])
                            else:
                                nu_sb = op.tile([P, D], F32, tag="nu_sb")
                                nc.vector.tensor_copy(nu_sb[:], nu_ps[:, 0:D])
                                nc.sync.dma_start(sink[1][rows, :], nu_sb[:])
                                se_sb = sp.tile([P, H], F32, tag="se_sb")
                                nc.vector.tensor_copy(se_sb[:], nu_ps[:, D:D + H])
                                nc.sync.dma_start(sink[2][rows, :], se_sb[:])

                pend = None
                for g0 in range(0, C, SUBB):
                    n = min(SUBB, C - g0)
                    kvg = gp.tile([P, SUBB, D2], BF16, tag="kvg", bufs=4)
                    nc.gpsimd.dma_gather(
                        out_ap=kvg[:, 0:n, :], in_ap=kv_tbl,
                        idxs_ap=src_sb[:, g0 * 8:(g0 + n) * 8],
                        num_idxs=n * P, num_idxs_reg=n * P, elem_size=D2)
                    ohg = ohp.tile([P, SUBB, 2 * P], BF16, tag="ohg")
                    nc.sync.dma_start(
                        ohg[:, 0:n, :],
                        oh_dram[g0:g0 + n].transpose([1, 0, 2]))
                    qk = sp.tile([P, SUBB, D], F32, tag="qk", bufs=3)
                    for j in range(n):
                        c = g0 + j
                        blk, first, last = info[c]
                        if first:
                            qblk = qbp.tile([P, D], BF16, tag="qblk")
                            nc.sync.dma_start(
                                qblk[:], q_tbl[blk * P:(blk + 1) * P, :])
                        qe = ps_qe.tile([P, D], F32, tag="qe")
                        nc.tensor.matmul(qe[:], lhsT=ohg[:, j, 0:P], rhs=qblk[:],
                                         start=True, stop=True)
                        nc.vector.tensor_tensor(
                            qk[:, j, :], qe[:], kvg[:, j, 0:D], ALU.mult)
                    s4 = sp.tile([P, SUBB, H], F32, tag="s4")
                    nc.vector.tensor_reduce(
                        s4[:, 0:n, :],
                        qk[:, 0:n, :].rearrange("p c (h k) -> p c h k", h=H),
                        axis=mybir.AxisListType.X, op=ALU.add)
                    ex0 = sp.tile([P, SUBB, H], F32, tag="ex0")
                    last_exp[0] = nc.scalar.activation(
                        ex0[:, 0:n, :], s4[:, 0:n, :], AF.Exp, scale=0.125)
                    exb = sp.tile([P, SUBB, H], F32, tag="exb")
                    nc.vector.tensor_tensor(
                        exb[:, 0:n, :], ex0[:, 0:n, :], nw_sb[:, g0:g0 + n, :],
                        ALU.mult)
                    exe = sp.tile([P, SUBB, D + H], BF16, tag="exe", bufs=3)
                    nc.vector.tensor_tensor(
                        exe[:, 0:n, 0:D].rearrange("p c (h k) -> p c h k", h=H),
                        kvg[:, 0:n, D:D2].rearrange("p c (h k) -> p c h k", h=H),
                        exb[:, 0:n, :].unsqueeze(-1).to_broadcast((P, n, H, DK)),
                        ALU.mult)
                    nc.vector.tensor_copy(exe[:, 0:n, D:D + H], exb[:, 0:n, :])
                    if pend is not None:
                        flush(pend)
                    pend = (g0, n, ohg, exe)
                if pend is not None:
                    flush(pend)
                for b, k in enumerate(meta):
                    if k != 0:
                        continue
                    rows = slice(b * P, (b + 1) * P)
                    if sink[0] == "msg":
                        nc.sync.dma_start(sink[1][rows, :], z256b[:])
                    else:
                        nc.sync.dma_start(sink[1][rows, :], z256[:])
                        nc.sync.dma_start(sink[2][rows, :], z4[:])

            d1_src_sb = wpool.tile([P, C1 * 8], I16, tag="d1_src_sb")
            nc.sync.dma_start(d1_src_sb[:], d1_src[:, :])
            d1_nw_sb = wpool.tile([P, C1, H], F32, tag="d1_nw_sb")
            nc.sync.dma_start(d1_nw_sb[:], d1_nw[:, :, :])
            d2_src_sb = wpool.tile([P, C2 * 8], I16, tag="d2_src_sb")
            nc.sync.dma_start(d2_src_sb[:], d2_src[:, :])
            d2_nw_sb = wpool.tile([P, C2, H], F32, tag="d2_nw_sb")
            nc.sync.dma_start(d2_nw_sb[:], d2_nw[:, :, :])

            phase_b(info1, C1, meta1, d1_src_sb, d1_nw_sb, d1_oh,
                    q_inv[:, :], kv_a_full[:, :], ("msg", msg_inv))
            phase_b(info2, C2, meta2, d2_src_sb, d2_nw_sb, d2_oh,
                    q_a_full[:, :], kv_inv[:, :], ("acc", numer_d, sumex_d))

            nc.gpsimd.collective_compute(
                "ReduceScatter", ALU.add, replica_groups=rg,
                ins=[numer_d.opt()], outs=[numer_sh.opt()])
            nc.gpsimd.collective_compute(
                "ReduceScatter", ALU.add, replica_groups=rg,
                ins=[sumex_d.opt()], outs=[sumex_sh.opt()])

            # ================= Phase C =================
            first_gelu = [None]

            def phase_c(ntiles, hT_in, msg_dram, out_dram):
                for t in range(ntiles):
                    cat = tp.tile([P, 4, P], BF16, tag="cat")
                    nc.sync.dma_start(cat[:, 0:2, :], hT_in[t * P:(t + 1) * P, :])
                    nc.sync.dma_start_transpose(
                        cat[:, 2, :], msg_dram[t * P:(t + 1) * P, 0:P])
                    nc.scalar.dma_start_transpose(
                        cat[:, 3, :], msg_dram[t * P:(t + 1) * P, P:D])
                    y1 = tp.tile([P, 2, P], BF16, tag="y1")
                    pm = ps_c.tile([P, D], F32, tag="pmc")
                    for hf in range(2):
                        for k in range(4):
                            nc.tensor.matmul(
                                pm[:, hf * P:(hf + 1) * P],
                                lhsT=u1_t[:, k, hf * P:(hf + 1) * P],
                                rhs=cat[:, k, :],
                                start=(k == 0), stop=(k == 3))
                    for hf in range(2):
                        g = nc.scalar.activation(
                            y1[:, hf, :], pm[:, hf * P:(hf + 1) * P], AF.Gelu,
                            bias=bu1_t[:, hf:hf + 1])
                        if first_gelu[0] is None:
                            first_gelu[0] = g
                    po = ps_c.tile([P, D], F32, tag="pmc")
                    for k in range(2):
                        nc.tensor.matmul(po[:], lhsT=y1[:, k, :],
                                         rhs=u2_t[:, k, :],
                                         start=(k == 0), stop=False)
                    nc.tensor.matmul(po[:], lhsT=ones_t[0:1, :],
                                     rhs=bu2_t[0:1, :], start=False, stop=True)
                    ot = op.tile([P, D], F32, tag="fin")
                    nc.scalar.activation(ot[:], po[:], AF.Gelu)
                    nc.sync.dma_start(out_dram[t * P:(t + 1) * P, :], ot[:])

            phase_c(NT_I, hT_i_in, msg_inv, out_inv)

            for t in range(NT_A):
                nu = hp.tile([P, D], F32, tag="nu_f")
                nc.sync.dma_start(nu[:], numer_sh[t * P:(t + 1) * P, :])
                se = sp.tile([P, H], F32, tag="se_f")
                nc.sync.dma_start(se[:], sumex_sh[t * P:(t + 1) * P, :])
                den = sp.tile([P, H], F32, tag="den")
                nc.vector.tensor_scalar(den[:], se[:], 1e-10, None, ALU.add)
                rec = sp.tile([P, H], F32, tag="rec")
                nc.vector.reciprocal(rec[:], den[:])
                msg = op.tile([P, D], BF16, tag="msg")
                nc.vector.tensor_tensor(
                    msg[:].rearrange("p (h k) -> p h k", h=H),
                    nu[:].rearrange("p (h k) -> p h k", h=H),
                    rec[:].unsqueeze(-1).to_broadcast((P, H, DK)), ALU.mult)
                nc.sync.dma_start(msg_ast[t * P:(t + 1) * P, :], msg[:])

            phase_c(NT_A, hT_a_in, msg_ast, out_ast)

    nc.compile()
    return nc


# ----------------------------------------------------------------------------
# Entry point
# ----------------------------------------------------------------------------

def kernel(inv_h, asset_h, inv_norm_w, asset_norm_w,
           m_w1, m_b1, m_w2, m_b2, Wq, Wk, Wv,
           u_w1, u_b1, u_w2, u_b2, edge_tgt, edge_src):
    global _LAST_EXEC_NS
    bf = ml_dtypes.bfloat16
    inv_h = np.asarray(inv_h, np.float32)
    asset_h = np.asarray(asset_h, np.float32)
    inv_norm_w = np.asarray(inv_norm_w, np.float32)
    asset_norm_w = np.asarray(asset_norm_w, np.float32)
    edge_tgt = np.asarray(edge_tgt).astype(np.int64)
    edge_src = np.asarray(edge_src).astype(np.int64)
    m_w1, m_b1 = np.asarray(m_w1, np.float32), np.asarray(m_b1, np.float32)
    m_w2, m_b2 = np.asarray(m_w2, np.float32), np.asarray(m_b2, np.float32)
    Wq, Wk, Wv = (np.asarray(x, np.float32) for x in (Wq, Wk, Wv))
    u_w1, u_b1 = np.asarray(u_w1, np.float32), np.asarray(u_b1, np.float32)
    u_w2, u_b2 = np.asarray(u_w2, np.float32), np.asarray(u_b2, np.float32)

    ast_row = (edge_src // ASH) * ASHP + (edge_src % ASH)
    inv_core = edge_tgt // ISH
    inv_loc = edge_tgt - inv_core * ISH

    meta1, C1, d1 = _bucket(
        core=inv_core, blk=inv_loc // P, n_blocks=NT_I,
        srcidx=ast_row, t128_all=inv_loc % P, nw=inv_norm_w)
    meta2, C2, d2 = _bucket(
        core=inv_core, blk=ast_row // P, n_blocks=NB2,
        srcidx=inv_loc, t128_all=ast_row % P, nw=asset_norm_w)

    nc = _build(meta1, C1, meta2, C2)

    w_qk_h = np.concatenate([Wq.reshape(2, P, D), Wk.reshape(2, P, D)], axis=2)
    common = {
        "w_m1": m_w1.reshape(2, P, D).astype(bf),
        "w_m2": m_w2.reshape(2, P, D).astype(bf),
        "w_qk": w_qk_h.astype(bf),
        "w_v": Wv.reshape(2, P, D).astype(bf),
        "w_u1": u_w1.reshape(4, P, D).astype(bf),
        "w_u2": u_w2.reshape(2, P, D).astype(bf),
        "b_m1": m_b1.reshape(2, P).T.copy(),
        "b_m2": m_b2.reshape(2, P).T.copy(),
        "b_u1": u_b1.reshape(2, P).T.copy(),
        "b_u2r": u_b2.reshape(1, D).astype(bf),
    }

    in_maps = []
    for c in range(NC):
        s1, t1, n1 = d1[c]
        s2, t2, n2 = d2[c]
        m = dict(common)
        m["hT_i_in"] = _hT(_pad_rows(inv_h[c * ISH:(c + 1) * ISH], ISHP), NT_I)
        m["hT_a_in"] = _hT(_pad_rows(asset_h[c * ASH:(c + 1) * ASH], ASHP), NT_A)
        m["d1_src"] = _wrap16(s1, C1)
        m["d1_nw"] = _colmajor(n1, C1, rep=H)
        m["d1_oh"] = _onehots(t1, C1)
        m["d2_src"] = _wrap16(s2, C2)
        m["d2_nw"] = _colmajor(n2, C2, rep=H)
        m["d2_oh"] = _onehots(t2, C2)
        in_maps.append(m)

    res = bass_utils.run_bass_kernel_spmd(
        nc, in_maps, core_ids=list(range(NC)), trace=True)
    _LAST_EXEC_NS = res.exec_time_ns

    inv_out = np.concatenate(
        [res.results[c]["out_inv"][:ISH] for c in range(NC)], axis=0)
    ast_out = np.concatenate(
        [res.results[c]["out_ast"][:ASH] for c in range(NC)], axis=0)
    return inv_out, ast_out
